# revision 1
# baseline (speedup 1.0000x reference)
"""CIGLoss (segment_reduce) Trainium2 kernel.

Strategy (data-parallel over batch, per the sharding hint):
  - Each of the 8 NeuronCores owns one image and that image's pixel list
    (segments are image-local: seg // 500 == image).
  - Host-side sharding packs each image's ~500 segments into a
    [128 partitions, NSLOT slots, L] padded grid (one whole segment per
    slot).  Pad entries point at a zero element appended to the image, so
    they contribute 0 to every sum.
  - The value lookup input[b,0,row,col] happens during host packing (this
    toolchain's walrus mis-lowers per-element indirect DMA: one descriptor
    per contiguous dest run, only the run-start offset honored — verified
    by hardware probes; see hw_gather_probe*.py).  All reductions run on
    device, per-slot:
        sums  = reduce_add(v)            counts = reduce_add(row < H)
        mean  = sums * recip(max(counts,1))
        dev   = reduce_add(|v - mean|)   contrib = dev * recip
    and a final partition reduce to one scalar per core.
  - Host sums the 8 per-core partials and divides by B.
"""

import numpy as np

_NUM_PATHS = 4000
_P = 128  # SBUF partitions


def _build_nc(nslot: int, L: int, ntot: int, W: int, H: int, chunk: int):
    import concourse.bacc as bacc
    import concourse.bass as bass
    import concourse.tile as tile
    from concourse import mybir

    f32 = mybir.dt.float32
    i32 = mybir.dt.int32
    Alu = mybir.AluOpType
    Ax = mybir.AxisListType
    FREE = nslot * L

    assert L % chunk == 0 or chunk % L == 0
    nch = FREE // chunk
    spc = max(1, chunk // L)   # whole slots per chunk (when chunk >= L)
    cps = max(1, L // chunk)   # chunks per slot (when chunk < L)

    u8 = mybir.dt.uint8
    nc = bacc.Bacc("TRN2", debug=False)
    v_d = nc.dram_tensor("vP", [_P, FREE], f32, kind="ExternalInput")
    ind_d = nc.dram_tensor("indP", [_P, FREE], u8, kind="ExternalInput")
    out_d = nc.dram_tensor("out", [_P, 1], f32, kind="ExternalOutput")

    _emit(nc, tile, bass, nslot, L, W, H, chunk, f32, u8, Alu, Ax,
          v_d, ind_d, out_d, FREE, nch, spc, cps)
    # Bacc defers register allocation + wait-splitting to finalize(); the
    # pjrt run path serializes the module as-is, so finalize here.
    nc.finalize()
    return nc


def _emit(nc, tile, bass, nslot, L, W, H, chunk, f32, u8, Alu, Ax,
          v_d, ind_d, out_d, FREE, nch, spc, cps):
    with tile.TileContext(nc) as tc:
        with (
            tc.tile_pool(name="big", bufs=1) as big,
            tc.tile_pool(name="small", bufs=1) as small,
        ):
            # u8 indicator of real (non-pad) pixels, upcast to f32
            ind8_t = big.tile([_P, FREE], u8)
            nc.sync.dma_start(out=ind8_t[:], in_=ind_d[:, :])
            ind_t = big.tile([_P, FREE], f32)
            nc.vector.tensor_copy(out=ind_t[:], in_=ind8_t[:])

            # gathered pixel values in slot layout; chunked load with
            # per-chunk partial sums so load and reduce overlap.
            v_t = big.tile([_P, FREE], f32)
            psum_t = small.tile([_P, nch * spc], f32)
            for k in range(nch):
                a, b = k * chunk, (k + 1) * chunk
                nc.sync.dma_start(out=v_t[:, a:b], in_=v_d[:, a:b])
                nc.vector.tensor_reduce(
                    out=psum_t[:, k * spc:(k + 1) * spc],
                    in_=v_t[:, a:b].rearrange("p (s l) -> p s l", s=spc),
                    axis=Ax.X, op=Alu.add,
                )

            v3 = v_t[:].rearrange("p (s l) -> p s l", s=nslot)
            ind3 = ind_t[:].rearrange("p (s l) -> p s l", s=nslot)

            # combine per-chunk partials into per-slot sums
            sums = small.tile([_P, nslot], f32)
            if cps == 1:
                nc.vector.tensor_copy(out=sums[:], in_=psum_t[:])
            elif cps == 2:
                nc.vector.tensor_tensor(
                    out=sums[:], in0=psum_t[:, 0::2], in1=psum_t[:, 1::2],
                    op=Alu.add,
                )
            else:
                nc.vector.tensor_reduce(
                    out=sums[:],
                    in_=psum_t[:].rearrange("p (s c) -> p s c", s=nslot),
                    axis=Ax.X, op=Alu.add,
                )
            counts = small.tile([_P, nslot], f32)
            nc.vector.tensor_reduce(out=counts[:], in_=ind3, axis=Ax.X, op=Alu.add)
            nc.vector.tensor_scalar_max(counts[:], counts[:], 1.0)
            w_t = small.tile([_P, nslot], f32)
            nc.vector.reciprocal(w_t[:], counts[:])
            means = small.tile([_P, nslot], f32)
            nc.vector.tensor_tensor(
                out=means[:], in0=sums[:], in1=w_t[:], op=Alu.mult
            )

            x_t = big.tile([_P, FREE], f32)
            x3 = x_t[:].rearrange("p (s l) -> p s l", s=nslot)
            nc.vector.tensor_tensor(
                out=x3, in0=v3, in1=means[:].to_broadcast([_P, nslot, L]),
                op=Alu.subtract,
            )
            devs = small.tile([_P, nslot], f32)
            nc.vector.tensor_reduce(
                out=devs[:], in_=x3, axis=Ax.X, op=Alu.add,
                apply_absolute_value=True,
            )
            # pads were gathered as 0, so each contributed |0 - mean| to devs;
            # subtract the known pad contribution (L - count) * |mean|.
            npad = small.tile([_P, nslot], f32)
            nc.vector.tensor_scalar(
                out=npad[:], in0=counts[:], scalar1=-1.0, scalar2=float(L),
                op0=Alu.mult, op1=Alu.add,
            )
            absm = small.tile([_P, nslot], f32)
            nc.vector.tensor_scalar(
                out=absm[:], in0=means[:], scalar1=-1.0, scalar2=None, op0=Alu.mult
            )
            nc.vector.tensor_tensor(
                out=absm[:], in0=absm[:], in1=means[:], op=Alu.max
            )
            nc.vector.tensor_tensor(
                out=npad[:], in0=npad[:], in1=absm[:], op=Alu.mult
            )
            nc.vector.tensor_tensor(
                out=devs[:], in0=devs[:], in1=npad[:], op=Alu.subtract
            )
            contrib = small.tile([_P, nslot], f32)
            nc.vector.tensor_tensor(
                out=contrib[:], in0=devs[:], in1=w_t[:], op=Alu.mult
            )
            part = small.tile([_P, 1], f32)
            nc.vector.tensor_reduce(
                out=part[:], in_=contrib[:], axis=Ax.X, op=Alu.add
            )
            nc.sync.dma_start(out=out_d[:, :], in_=part[:])
    return nc


_CACHE = {}


def _get_nc(key):
    if key not in _CACHE:
        _CACHE[key] = _build_nc(*key)
    return _CACHE[key]


def _pack(input, rows, cols, seg_ids, num_paths):
    """Host-side sharding: one image per core, segments packed into a
    [ncore, 128, nslot*L] padded slot grid."""
    B, C, H, W = input.shape
    ppi = num_paths // B  # paths (segments) per image
    npix = rows.shape[0]

    bnd = np.searchsorted(seg_ids, np.arange(num_paths + 1)).astype(np.int64)
    seg_lens = np.diff(bnd)
    nslot = int(np.ceil(ppi / _P))
    lmax = int(seg_lens.max()) if npix else 1
    L = max(128, int(np.ceil(lmax / 128.0)) * 128)
    FREE = nslot * L

    s = np.arange(num_paths)
    core = s // ppi
    local = s % ppi
    part = local % _P
    slot = local // _P
    base = ((core * _P + part) * np.int64(nslot) + slot) * L
    dest = np.repeat(base, seg_lens) + (
        np.arange(npix, dtype=np.int64) - np.repeat(bnd[:-1], seg_lens)
    )
    ind_p = np.zeros(B * _P * FREE, np.uint8)
    ind_p[dest] = 1
    # Pixel values in slot layout.  This lookup runs on the host: the
    # toolchain's walrus build mis-lowers sub-row indirect DMA (one
    # descriptor per contiguous dest run, only the run-start offset is
    # honored), so a per-element device gather is not expressible; all
    # reductions stay on device.
    core_of = np.repeat(core, seg_lens)
    v_p = np.zeros(B * _P * FREE, np.float32)
    v_p[dest] = input[core_of, 0, rows, cols]
    return (v_p.reshape(B, _P, FREE), ind_p.reshape(B, _P, FREE),
            nslot, L, H * W + 128)


def kernel(input, rows, cols, seg_ids, _trace=False, _num_paths=_NUM_PATHS):
    from concourse.bass_utils import run_bass_kernel_spmd

    input = np.ascontiguousarray(np.asarray(input, np.float32))
    rows = np.ascontiguousarray(np.asarray(rows, np.int32))
    cols = np.ascontiguousarray(np.asarray(cols, np.int32))
    seg_ids = np.ascontiguousarray(np.asarray(seg_ids, np.int32))
    B, C, H, W = input.shape

    v_p, ind_p, nslot, L, ntot = _pack(input, rows, cols, seg_ids, _num_paths)
    chunk = L // 2 if (L % 2 == 0 and L >= 512) else L
    nc = _get_nc((nslot, L, ntot, W, H, chunk))
    in_maps = [
        {"vP": v_p[i], "indP": ind_p[i]} for i in range(B)
    ]
    res = run_bass_kernel_spmd(nc, in_maps, core_ids=list(range(B)), trace=_trace)
    total = sum(float(r["out"].sum()) for r in res.results)
    out = np.float32(total / B)
    if _trace:
        return out, res
    return out



# revision 11
# speedup vs baseline: 1.8804x; 1.8804x over previous
"""CIGLoss (segment_reduce) Trainium2 kernel.

Strategy (data-parallel over batch, per the sharding hint):
  - Each of the 8 NeuronCores owns one image and that image's pixel list
    (segments are image-local: seg // 500 == image).
  - Host-side packing places each image's 500 segments into a
    [128 partitions, 4 slots] grid, one whole segment per (partition,
    slot) row, sorted by length so slot k only needs Lk elements;
    pads are zeros.  Values are bf16 (tolerance is 2e-2; bf16 keeps the
    DVE in its fast packed modes and halves HBM traffic).
  - The value lookup input[b,0,row,col] happens during host packing
    (walrus mis-lowers per-element indirect DMA, so a device-side
    gather is not expressible).  All reductions run on device:
      sums_k : tensor_scalar(mult 1, reduce-add accum)       [DVE 4x]
      mean_k : sums * recip(count)                           [DVE]
      dev_k  : sum|v - m| = sum max(v,m) - sum min(v,m)
               (the L*m terms cancel; pads contribute |m|, corrected
               via the precomputed w2 = npad*recip weights)
               first nact slots on the scalar engine as
               ACT(Abs, scale=-1, bias=m, accum); the rest as two
               tensor_scalar max/min reduce-accums on DVE
      final  : contrib = recip*(dev - w2*|sums|*recip ...) reduced over
               slots, then a 128-partition reduce via PE matmul with a
               ones column
  - Output is a single [1,1] f32 per core (single-packet DMA); the host
    sums the 8 per-core partials and divides by B.
"""

import numpy as np

_NUM_PATHS = 4000
_P = 128  # SBUF partitions
_NACT = 4  # slots whose dev pass runs on the scalar engine (rest on DVE)


def _build_nc(Ls, nact):
    import concourse.bacc as bacc
    import concourse.tile as tile
    from concourse import mybir

    f32 = mybir.dt.float32
    bf16 = mybir.dt.bfloat16
    Alu = mybir.AluOpType
    Ax = mybir.AxisListType
    Act = mybir.ActivationFunctionType

    nslot = len(Ls)
    offs = [sum(Ls[:k]) for k in range(nslot)]
    FREE = sum(Ls)
    Lmax = max(Ls)

    nc = bacc.Bacc("TRN2", debug=False)
    v_d = nc.dram_tensor("vP", [_P, FREE], bf16, kind="ExternalInput")
    meta_d = nc.dram_tensor("meta", [_P, 16], f32, kind="ExternalInput")
    out_d = nc.dram_tensor("out", [1, 1], f32, kind="ExternalOutput")

    with tile.TileContext(nc) as tc:
        with (
            tc.tile_pool(name="pool", bufs=1) as pool,
            tc.tile_pool(name="ps", bufs=1, space="PSUM") as ps,
        ):
            meta = pool.tile([_P, 16], f32)
            nc.sync.dma_start(out=meta[:], in_=meta_d[:, :])
            recip = meta[:, 0:4]
            w2 = meta[:, 4:8]
            ones = meta[:, 8:9]

            # spread the input DMA kicks across engine queues: each kick
            # costs ~0.6us of queue time, serializing them on sync alone
            # delays the last slot's data by ~2.4us
            kick = [nc.sync, nc.gpsimd, nc.scalar, nc.gpsimd]
            v = pool.tile([_P, FREE], bf16)
            for k in range(nslot):
                a, b = offs[k], offs[k] + Ls[k]
                kick[k % len(kick)].dma_start(out=v[:, a:b], in_=v_d[:, a:b])

            scr = pool.tile([_P, Lmax], bf16)    # DVE scratch
            scr2 = pool.tile([_P, Lmax], bf16)   # ACT scratch
            sums = pool.tile([_P, nslot], f32)
            mpos = pool.tile([_P, nslot], f32)
            devs = pool.tile([_P, nslot], f32)
            small = pool.tile([_P, 10], f32)
            if nact < nslot:
                dmin = pool.tile([_P, nslot], f32)
                nc.vector.memset(dmin[:], 0.0)

            for k in range(nslot):
                a, b = offs[k], offs[k] + Ls[k]
                nc.vector.tensor_scalar(
                    out=scr[:, 0:Ls[k]], in0=v[:, a:b], scalar1=1.0,
                    scalar2=None, op0=Alu.mult, op1=Alu.add,
                    accum_out=sums[:, k:k + 1])
                nc.vector.tensor_tensor(
                    out=mpos[:, k:k + 1], in0=sums[:, k:k + 1],
                    in1=recip[:, k:k + 1], op=Alu.mult)
                if k < nact:
                    # |v - m| = Abs(-v + m): scale=-1, bias=m
                    nc.scalar.activation(
                        out=scr2[:, 0:Ls[k]], in_=v[:, a:b], func=Act.Abs,
                        bias=mpos[:, k:k + 1], scale=-1.0,
                        accum_out=devs[:, k:k + 1])
                else:
                    nc.vector.tensor_scalar(
                        out=scr[:, 0:Ls[k]], in0=v[:, a:b],
                        scalar1=mpos[:, k:k + 1], scalar2=None,
                        op0=Alu.max, op1=Alu.add,
                        accum_out=devs[:, k:k + 1])
                    nc.vector.tensor_scalar(
                        out=scr[:, 0:Ls[k]], in0=v[:, a:b],
                        scalar1=mpos[:, k:k + 1], scalar2=None,
                        op0=Alu.min, op1=Alu.add,
                        accum_out=dmin[:, k:k + 1])

            # contrib = recip * (dev_raw - w2*|m|), w2 = npad
            # |m| via max/min pair on mpos (these only need mpos, so the
            # scheduler can run them under the trailing ACT slots)
            sa = small[:, 0:4]
            sb = small[:, 4:8]
            nc.vector.tensor_scalar(
                out=sa, in0=mpos[:], scalar1=0.0, scalar2=None, op0=Alu.max)
            nc.vector.tensor_scalar(
                out=sb, in0=mpos[:], scalar1=0.0, scalar2=None, op0=Alu.min)
            nc.vector.tensor_tensor(out=sa, in0=sa, in1=sb, op=Alu.subtract)
            # sa = |m|
            nc.vector.tensor_tensor(out=sa, in0=w2, in1=sa, op=Alu.mult)
            if nact < nslot:
                nc.vector.tensor_tensor(out=devs[:], in0=devs[:],
                                        in1=dmin[:], op=Alu.subtract)
            nc.vector.tensor_tensor(out=devs[:], in0=devs[:], in1=sa,
                                    op=Alu.subtract)
            nc.vector.tensor_tensor(out=devs[:], in0=devs[:], in1=recip,
                                    op=Alu.mult)
            tot = small[:, 8:9]
            nc.vector.tensor_reduce(out=tot, in_=devs[:], axis=Ax.X,
                                    op=Alu.add)

            pt = ps.tile([1, 1], f32)
            nc.tensor.matmul(pt[:], ones, tot)
            osc = pool.tile([1, 1], f32)
            nc.vector.tensor_copy(out=osc[:], in_=pt[:])
            nc.sync.dma_start(out=out_d[:, :], in_=osc[:], single_packet=True)
    nc.finalize()
    return nc


_CACHE = {}


def _get_nc(key):
    if key not in _CACHE:
        _CACHE[key] = _build_nc(*key)
    return _CACHE[key]


def _pack(input, rows, cols, seg_ids, num_paths):
    """Host-side sharding: one image per core; segments sorted by length
    into a [128, nslot] slot grid with per-slot lengths Lk."""
    import ml_dtypes

    B, C, H, W = input.shape
    ppi = num_paths // B
    npix = rows.shape[0]
    nslot = (ppi + _P - 1) // _P

    bnd = np.searchsorted(seg_ids, np.arange(num_paths + 1)).astype(np.int64)
    seg_lens = np.diff(bnd)  # [num_paths]
    lens2 = seg_lens.reshape(B, ppi)

    # per-core rank by descending length -> (slot, partition)
    order = np.argsort(-lens2, axis=1, kind="stable")  # [B, ppi]
    rank = np.empty_like(order)
    np.put_along_axis(rank, order, np.arange(ppi)[None, :].repeat(B, 0), 1)
    slot = rank // _P          # [B, ppi]
    part = rank % _P

    # per-slot max length over all cores, rounded up to multiple of 8
    slot_max = np.zeros(nslot, np.int64)
    for k in range(nslot):
        m = lens2[slot == k]
        if m.size:
            slot_max[k] = m.max()
    Ls = tuple(int(max(256, -(-int(l) // 8) * 8)) for l in slot_max)
    offs = np.concatenate([[0], np.cumsum(Ls)]).astype(np.int64)
    FREE = int(offs[-1])

    # destination index for every pixel
    core_of_seg = np.repeat(np.arange(B), ppi)
    base = (core_of_seg * _P + part.ravel()) * np.int64(FREE) \
        + offs[:-1][slot.ravel()]
    dest = np.repeat(base, seg_lens) + (
        np.arange(npix, dtype=np.int64) - np.repeat(bnd[:-1], seg_lens)
    )
    vals = input[np.repeat(core_of_seg, seg_lens), 0, rows, cols]
    v_p = np.zeros(B * _P * FREE, np.float32)
    v_p[dest] = vals
    v_p = v_p.reshape(B, _P, FREE).astype(ml_dtypes.bfloat16)

    # meta: recip [0:4], w2 [4:8], ones col 8
    cnt = np.zeros((B, _P, nslot), np.float64)
    for b in range(B):
        cnt[b, part[b], slot[b]] = lens2[b]
    cmax = np.maximum(cnt, 1.0)
    recip = 1.0 / cmax
    w2 = np.asarray(Ls)[None, None, :] - cnt  # npad per (partition, slot)
    meta = np.zeros((B, _P, 16), np.float32)
    meta[:, :, 0:nslot] = recip
    meta[:, :, 4:4 + nslot] = w2
    meta[:, :, 8] = 1.0
    return v_p, meta, Ls


def kernel(input, rows, cols, seg_ids, _trace=False, _num_paths=_NUM_PATHS,
           _nact=_NACT):
    from concourse.bass_utils import run_bass_kernel_spmd

    input = np.ascontiguousarray(np.asarray(input, np.float32))
    rows = np.ascontiguousarray(np.asarray(rows, np.int32))
    cols = np.ascontiguousarray(np.asarray(cols, np.int32))
    seg_ids = np.ascontiguousarray(np.asarray(seg_ids, np.int32))
    B = input.shape[0]

    v_p, meta, Ls = _pack(input, rows, cols, seg_ids, _num_paths)
    nc = _get_nc((Ls, _nact))
    in_maps = [{"vP": v_p[i], "meta": meta[i]} for i in range(B)]
    res = run_bass_kernel_spmd(nc, in_maps, core_ids=list(range(B)),
                               trace=_trace)
    total = sum(float(r["out"][0, 0]) for r in res.results)
    out = np.float32(total / B)
    if _trace:
        return out, res
    return out


# revision 16
# speedup vs baseline: 1.9106x; 1.0160x over previous
"""CIGLoss (segment_reduce) Trainium2 kernel.

Strategy (data-parallel over batch, per the sharding hint):
  - Each of the 8 NeuronCores owns one image and that image's pixel list
    (segments are image-local: seg // 500 == image).
  - Host-side packing places each image's 500 segments into a
    [128 partitions, 4 slots] grid, one whole segment per (partition,
    slot) row, sorted by length so slot k only needs Lk elements;
    pads are zeros.  Values are bf16 (tolerance is 2e-2; bf16 keeps the
    DVE in its fast packed modes and halves HBM traffic).
  - The value lookup input[b,0,row,col] happens during host packing
    (walrus mis-lowers per-element indirect DMA, so a device-side
    gather is not expressible).  All reductions run on device:
      sums_k : tensor_scalar(mult 1, reduce-add accum)       [DVE 4x]
      mean_k : sums * recip(count)                           [DVE]
      dev_k  : sum|v - m| = sum max(v,m) - sum min(v,m)
               (the L*m terms cancel; pads contribute |m|, corrected
               via the precomputed w2 = npad*recip weights)
               first nact slots on the scalar engine as
               ACT(Abs, scale=-1, bias=m, accum); the rest as two
               tensor_scalar max/min reduce-accums on DVE
      final  : contrib = recip*(dev - w2*|sums|*recip ...) reduced over
               slots, then a 128-partition reduce via PE matmul with a
               ones column
  - Output is a single [1,1] f32 per core (single-packet DMA); the host
    sums the 8 per-core partials and divides by B.
"""

import numpy as np

_NUM_PATHS = 4000
_P = 128  # SBUF partitions
_NACT = 4  # slots whose dev pass runs on the scalar engine (rest on DVE)


def _build_nc(Ls, nact):
    import concourse.bacc as bacc
    import concourse.tile as tile
    from concourse import mybir

    f32 = mybir.dt.float32
    fp8 = mybir.dt.float8e4
    Alu = mybir.AluOpType
    Ax = mybir.AxisListType
    Act = mybir.ActivationFunctionType

    nslot = len(Ls)
    offs = [sum(Ls[:k]) for k in range(nslot)]
    FREE = sum(Ls)
    Lmax = max(Ls)

    nc = bacc.Bacc("TRN2", debug=False)
    v_d = nc.dram_tensor("vP", [_P, FREE], fp8, kind="ExternalInput")
    meta_d = nc.dram_tensor("meta", [_P, 16], f32, kind="ExternalInput")
    out_d = nc.dram_tensor("out", [1, 1], f32, kind="ExternalOutput")

    with tile.TileContext(nc) as tc:
        with (
            tc.tile_pool(name="pool", bufs=1) as pool,
            tc.tile_pool(name="ps", bufs=1, space="PSUM") as ps,
        ):
            meta = pool.tile([_P, 16], f32)
            recip = meta[:, 0:4]
            w2 = meta[:, 4:8]
            ones = meta[:, 8:9]
            negones = meta[:, 9:10]

            # spread the input DMA kicks across engine queues (each kick
            # costs ~0.7us of queue time); slot 0 goes first, on the
            # scalar queue, so its consumer chain starts soonest
            v = pool.tile([_P, FREE], fp8)
            kick = [nc.scalar, nc.gpsimd, nc.sync, nc.gpsimd]
            for k in range(nslot):
                a, b = offs[k], offs[k] + Ls[k]
                kick[k % len(kick)].dma_start(out=v[:, a:b], in_=v_d[:, a:b])
            nc.sync.dma_start(out=meta[:], in_=meta_d[:, :])

            scr = pool.tile([_P, Lmax], fp8)     # DVE scratch
            scr2 = pool.tile([_P, Lmax], fp8)    # ACT scratch
            sums = pool.tile([_P, nslot], f32)
            mpos = pool.tile([_P, nslot], f32)
            devs = pool.tile([_P, nslot], f32)
            small = pool.tile([_P, 10], f32)
            if nact < nslot:
                dmin = pool.tile([_P, nslot], f32)
                nc.vector.memset(dmin[:], 0.0)

            for k in range(nslot):
                a, b = offs[k], offs[k] + Ls[k]
                nc.vector.tensor_scalar(
                    out=scr[:, 0:Ls[k]], in0=v[:, a:b], scalar1=1.0,
                    scalar2=None, op0=Alu.mult, op1=Alu.add,
                    accum_out=sums[:, k:k + 1])
                nc.vector.tensor_tensor(
                    out=mpos[:, k:k + 1], in0=sums[:, k:k + 1],
                    in1=recip[:, k:k + 1], op=Alu.mult)
                if k < nact:
                    # |v - m| = Abs(-v + m): scale=-1, bias=m
                    nc.scalar.activation(
                        out=scr2[:, 0:Ls[k]], in_=v[:, a:b], func=Act.Abs,
                        bias=mpos[:, k:k + 1], scale=-1.0,
                        accum_out=devs[:, k:k + 1])
                else:
                    nc.vector.tensor_scalar(
                        out=scr[:, 0:Ls[k]], in0=v[:, a:b],
                        scalar1=mpos[:, k:k + 1], scalar2=None,
                        op0=Alu.max, op1=Alu.add,
                        accum_out=devs[:, k:k + 1])
                    nc.vector.tensor_scalar(
                        out=scr[:, 0:Ls[k]], in0=v[:, a:b],
                        scalar1=mpos[:, k:k + 1], scalar2=None,
                        op0=Alu.min, op1=Alu.add,
                        accum_out=dmin[:, k:k + 1])

            # loss = sum_k recip*dev_raw  -  sum_k recip*w2*|m|
            # the correction term only needs mpos, so it runs under the
            # trailing ACT slots; the two terms meet in an accumulating
            # matmul pair (+ones, -ones) on the PE
            sa = small[:, 0:4]
            sb = small[:, 4:8]
            nc.vector.tensor_scalar(
                out=sa, in0=mpos[:], scalar1=0.0, scalar2=None, op0=Alu.max)
            nc.vector.tensor_scalar(
                out=sb, in0=mpos[:], scalar1=0.0, scalar2=None, op0=Alu.min)
            nc.vector.tensor_tensor(out=sa, in0=sa, in1=sb, op=Alu.subtract)
            # sa = |m|
            nc.vector.tensor_tensor(out=sa, in0=w2, in1=sa, op=Alu.mult)
            nc.vector.tensor_tensor(out=sa, in0=sa, in1=recip, op=Alu.mult)
            corr = small[:, 8:9]
            nc.vector.tensor_reduce(out=corr, in_=sa, axis=Ax.X, op=Alu.add)

            if nact < nslot:
                nc.vector.tensor_tensor(out=devs[:], in0=devs[:],
                                        in1=dmin[:], op=Alu.subtract)
            nc.vector.tensor_tensor(out=devs[:], in0=devs[:], in1=recip,
                                    op=Alu.mult)
            tot = small[:, 9:10]
            nc.vector.tensor_reduce(out=tot, in_=devs[:], axis=Ax.X,
                                    op=Alu.add)

            pt = ps.tile([1, 1], f32)
            nc.tensor.matmul(pt[:], ones, tot, start=True, stop=False)
            nc.tensor.matmul(pt[:], negones, corr, start=False, stop=True)
            osc = pool.tile([1, 1], f32)
            nc.vector.tensor_copy(out=osc[:], in_=pt[:])
            nc.sync.dma_start(out=out_d[:, :], in_=osc[:], single_packet=True)
    nc.finalize()
    return nc


_CACHE = {}


def _get_nc(key):
    if key not in _CACHE:
        _CACHE[key] = _build_nc(*key)
    return _CACHE[key]


def _pack(input, rows, cols, seg_ids, num_paths):
    """Host-side sharding: one image per core; segments sorted by length
    into a [128, nslot] slot grid with per-slot lengths Lk."""
    import ml_dtypes

    B, C, H, W = input.shape
    ppi = num_paths // B
    npix = rows.shape[0]
    nslot = (ppi + _P - 1) // _P

    bnd = np.searchsorted(seg_ids, np.arange(num_paths + 1)).astype(np.int64)
    seg_lens = np.diff(bnd)  # [num_paths]
    lens2 = seg_lens.reshape(B, ppi)

    # per-core rank by descending length -> (slot, partition)
    order = np.argsort(-lens2, axis=1, kind="stable")  # [B, ppi]
    rank = np.empty_like(order)
    np.put_along_axis(rank, order, np.arange(ppi)[None, :].repeat(B, 0), 1)
    slot = rank // _P          # [B, ppi]
    part = rank % _P

    # per-slot max length over all cores, rounded up to multiple of 8
    slot_max = np.zeros(nslot, np.int64)
    for k in range(nslot):
        m = lens2[slot == k]
        if m.size:
            slot_max[k] = m.max()
    Ls = tuple(int(max(256, -(-int(l) // 8) * 8)) for l in slot_max)
    offs = np.concatenate([[0], np.cumsum(Ls)]).astype(np.int64)
    FREE = int(offs[-1])

    # destination index for every pixel
    core_of_seg = np.repeat(np.arange(B), ppi)
    base = (core_of_seg * _P + part.ravel()) * np.int64(FREE) \
        + offs[:-1][slot.ravel()]
    dest = np.repeat(base, seg_lens) + (
        np.arange(npix, dtype=np.int64) - np.repeat(bnd[:-1], seg_lens)
    )
    vals = input[np.repeat(core_of_seg, seg_lens), 0, rows, cols]
    v_p = np.zeros(B * _P * FREE, np.float32)
    v_p[dest] = vals
    v_p = v_p.reshape(B, _P, FREE).astype(ml_dtypes.float8_e4m3)

    # meta: recip [0:4], w2 [4:8], ones col 8, -ones col 9
    cnt = np.zeros((B, _P, nslot), np.float64)
    for b in range(B):
        cnt[b, part[b], slot[b]] = lens2[b]
    cmax = np.maximum(cnt, 1.0)
    recip = 1.0 / cmax
    w2 = np.asarray(Ls)[None, None, :] - cnt  # npad per (partition, slot)
    meta = np.zeros((B, _P, 16), np.float32)
    meta[:, :, 0:nslot] = recip
    meta[:, :, 4:4 + nslot] = w2
    meta[:, :, 8] = 1.0
    meta[:, :, 9] = -1.0
    return v_p, meta, Ls


def kernel(input, rows, cols, seg_ids, _trace=False, _num_paths=_NUM_PATHS,
           _nact=_NACT):
    from concourse.bass_utils import run_bass_kernel_spmd

    input = np.ascontiguousarray(np.asarray(input, np.float32))
    rows = np.ascontiguousarray(np.asarray(rows, np.int32))
    cols = np.ascontiguousarray(np.asarray(cols, np.int32))
    seg_ids = np.ascontiguousarray(np.asarray(seg_ids, np.int32))
    B = input.shape[0]

    v_p, meta, Ls = _pack(input, rows, cols, seg_ids, _num_paths)
    nc = _get_nc((Ls, _nact))
    in_maps = [{"vP": v_p[i], "meta": meta[i]} for i in range(B)]
    res = run_bass_kernel_spmd(nc, in_maps, core_ids=list(range(B)),
                               trace=_trace)
    total = sum(float(r["out"][0, 0]) for r in res.results)
    out = np.float32(total / B)
    if _trace:
        return out, res
    return out


# revision 18
# speedup vs baseline: 1.9922x; 1.0427x over previous
"""CIGLoss (segment_reduce) Trainium2 kernel.

Strategy (data-parallel over batch, per the sharding hint):
  - Each of the 8 NeuronCores owns one image and that image's pixel list
    (segments are image-local: seg // 500 == image).
  - Host-side packing places each image's 500 segments into a
    [128 partitions, 4 slots] grid, one whole segment per (partition,
    slot) row, sorted by length so slot k only needs Lk elements;
    pads are zeros.  Values are bf16 (tolerance is 2e-2; bf16 keeps the
    DVE in its fast packed modes and halves HBM traffic).
  - The value lookup input[b,0,row,col] happens during host packing
    (walrus mis-lowers per-element indirect DMA, so a device-side
    gather is not expressible).  All reductions run on device:
      sums_k : tensor_scalar(mult 1, reduce-add accum)       [DVE 4x]
      mean_k : sums * recip(count)                           [DVE]
      dev_k  : sum|v - m| = sum max(v,m) - sum min(v,m)
               (the L*m terms cancel; pads contribute |m|, corrected
               via the precomputed w2 = npad*recip weights)
               first nact slots on the scalar engine as
               ACT(Abs, scale=-1, bias=m, accum); the rest as two
               tensor_scalar max/min reduce-accums on DVE
      final  : contrib = recip*(dev - w2*|sums|*recip ...) reduced over
               slots, then a 128-partition reduce via PE matmul with a
               ones column
  - Output is a single [1,1] f32 per core (single-packet DMA); the host
    sums the 8 per-core partials and divides by B.
"""

import numpy as np

_NUM_PATHS = 4000
_P = 128  # SBUF partitions
_NACT = 4  # slots whose dev pass runs on the scalar engine (rest on DVE)


def _build_nc(Ls, nact):
    import concourse.bacc as bacc
    import concourse.tile as tile
    from concourse import mybir

    f32 = mybir.dt.float32
    fp8 = mybir.dt.float8e4
    Alu = mybir.AluOpType
    Ax = mybir.AxisListType
    Act = mybir.ActivationFunctionType

    nslot = len(Ls)
    offs = [sum(Ls[:k]) for k in range(nslot)]
    FREE = sum(Ls)
    Lmax = max(Ls)

    nc = bacc.Bacc("TRN2", debug=False)
    v_d = nc.dram_tensor("vP", [_P, FREE], fp8, kind="ExternalInput")
    meta_d = nc.dram_tensor("meta", [_P, 16], f32, kind="ExternalInput")
    out_d = nc.dram_tensor("out", [1, 1], f32, kind="ExternalOutput")

    with tile.TileContext(nc) as tc:
        with (
            tc.tile_pool(name="pool", bufs=1) as pool,
            tc.tile_pool(name="ps", bufs=1, space="PSUM") as ps,
        ):
            meta = pool.tile([_P, 16], f32)
            recip = meta[:, 0:4]
            w2 = meta[:, 4:8]
            ones = meta[:, 8:9]
            negones = meta[:, 9:10]

            # Input DMA layout: DGE assigns contiguous 8-row chunks to the
            # 16 hw rings, and ring 15 (E79) consistently starts ~2us after
            # the rest, delaying every 128-row DMA's completion semaphore.
            # So each slot's main DMA covers partitions [0:120] (15 fast
            # rings only) and one combined DMA, kicked first, carries all
            # slots' [120:128] tail rows.  Kicks are spread across the
            # three DMA-capable engine queues (~0.7us of queue time each).
            v = pool.tile([_P, FREE], fp8)
            nc.sync.dma_start(out=v[120:128, :], in_=v_d[120:128, :])
            kick = [nc.scalar, nc.sync, nc.gpsimd, nc.gpsimd]
            for k in range(nslot):
                a, b = offs[k], offs[k] + Ls[k]
                kick[k].dma_start(out=v[0:120, a:b], in_=v_d[0:120, a:b])
            nc.scalar.dma_start(out=meta[:], in_=meta_d[:, :])

            scr = pool.tile([_P, Lmax], fp8)     # DVE scratch
            scr2 = pool.tile([_P, Lmax], fp8)    # ACT scratch
            sums = pool.tile([_P, nslot], f32)
            mpos = pool.tile([_P, nslot], f32)
            devs = pool.tile([_P, nslot], f32)
            small = pool.tile([_P, 10], f32)
            if nact < nslot:
                dmin = pool.tile([_P, nslot], f32)
                nc.vector.memset(dmin[:], 0.0)

            for k in range(nslot):
                a, b = offs[k], offs[k] + Ls[k]
                nc.vector.tensor_scalar(
                    out=scr[:, 0:Ls[k]], in0=v[:, a:b], scalar1=1.0,
                    scalar2=None, op0=Alu.mult, op1=Alu.add,
                    accum_out=sums[:, k:k + 1])
                nc.vector.tensor_tensor(
                    out=mpos[:, k:k + 1], in0=sums[:, k:k + 1],
                    in1=recip[:, k:k + 1], op=Alu.mult)
                if k < nact:
                    # |v - m| = Abs(-v + m): scale=-1, bias=m
                    nc.scalar.activation(
                        out=scr2[:, 0:Ls[k]], in_=v[:, a:b], func=Act.Abs,
                        bias=mpos[:, k:k + 1], scale=-1.0,
                        accum_out=devs[:, k:k + 1])
                else:
                    nc.vector.tensor_scalar(
                        out=scr[:, 0:Ls[k]], in0=v[:, a:b],
                        scalar1=mpos[:, k:k + 1], scalar2=None,
                        op0=Alu.max, op1=Alu.add,
                        accum_out=devs[:, k:k + 1])
                    nc.vector.tensor_scalar(
                        out=scr[:, 0:Ls[k]], in0=v[:, a:b],
                        scalar1=mpos[:, k:k + 1], scalar2=None,
                        op0=Alu.min, op1=Alu.add,
                        accum_out=dmin[:, k:k + 1])

            # loss = sum_k recip*dev_raw  -  sum_k recip*w2*|m|
            # the correction term only needs mpos, so it runs under the
            # trailing ACT slots; the two terms meet in an accumulating
            # matmul pair (+ones, -ones) on the PE
            sa = small[:, 0:4]
            sb = small[:, 4:8]
            nc.vector.tensor_scalar(
                out=sa, in0=mpos[:], scalar1=0.0, scalar2=None, op0=Alu.max)
            nc.vector.tensor_scalar(
                out=sb, in0=mpos[:], scalar1=0.0, scalar2=None, op0=Alu.min)
            nc.vector.tensor_tensor(out=sa, in0=sa, in1=sb, op=Alu.subtract)
            # sa = |m|
            nc.vector.tensor_tensor(out=sa, in0=w2, in1=sa, op=Alu.mult)
            nc.vector.tensor_tensor(out=sa, in0=sa, in1=recip, op=Alu.mult)
            corr = small[:, 8:9]
            nc.vector.tensor_reduce(out=corr, in_=sa, axis=Ax.X, op=Alu.add)

            if nact < nslot:
                nc.vector.tensor_tensor(out=devs[:], in0=devs[:],
                                        in1=dmin[:], op=Alu.subtract)
            nc.vector.tensor_tensor(out=devs[:], in0=devs[:], in1=recip,
                                    op=Alu.mult)
            tot = small[:, 9:10]
            nc.vector.tensor_reduce(out=tot, in_=devs[:], axis=Ax.X,
                                    op=Alu.add)

            pt = ps.tile([1, 1], f32)
            nc.tensor.matmul(pt[:], ones, tot, start=True, stop=False)
            nc.tensor.matmul(pt[:], negones, corr, start=False, stop=True)
            osc = pool.tile([1, 1], f32)
            nc.vector.tensor_copy(out=osc[:], in_=pt[:])
            nc.sync.dma_start(out=out_d[:, :], in_=osc[:], single_packet=True)
    nc.finalize()
    return nc


_CACHE = {}


def _get_nc(key):
    if key not in _CACHE:
        _CACHE[key] = _build_nc(*key)
    return _CACHE[key]


def _pack(input, rows, cols, seg_ids, num_paths):
    """Host-side sharding: one image per core; segments sorted by length
    into a [128, nslot] slot grid with per-slot lengths Lk."""
    import ml_dtypes

    B, C, H, W = input.shape
    ppi = num_paths // B
    npix = rows.shape[0]
    nslot = (ppi + _P - 1) // _P

    bnd = np.searchsorted(seg_ids, np.arange(num_paths + 1)).astype(np.int64)
    seg_lens = np.diff(bnd)  # [num_paths]
    lens2 = seg_lens.reshape(B, ppi)

    # per-core rank by descending length -> (slot, partition); the
    # shortest (partial) block becomes slot 0 so the first sums pass is
    # quick and the ACT chain starts sooner
    order = np.argsort(-lens2, axis=1, kind="stable")  # [B, ppi]
    rank = np.empty_like(order)
    np.put_along_axis(rank, order, np.arange(ppi)[None, :].repeat(B, 0), 1)
    slot = (rank // _P + 1) % nslot   # [B, ppi]
    part = rank % _P

    # per-slot max length over all cores, rounded up to multiple of 8
    slot_max = np.zeros(nslot, np.int64)
    for k in range(nslot):
        m = lens2[slot == k]
        if m.size:
            slot_max[k] = m.max()
    Ls = tuple(int(max(256, -(-int(l) // 8) * 8)) for l in slot_max)
    offs = np.concatenate([[0], np.cumsum(Ls)]).astype(np.int64)
    FREE = int(offs[-1])

    # destination index for every pixel
    core_of_seg = np.repeat(np.arange(B), ppi)
    base = (core_of_seg * _P + part.ravel()) * np.int64(FREE) \
        + offs[:-1][slot.ravel()]
    dest = np.repeat(base, seg_lens) + (
        np.arange(npix, dtype=np.int64) - np.repeat(bnd[:-1], seg_lens)
    )
    vals = input[np.repeat(core_of_seg, seg_lens), 0, rows, cols]
    v_p = np.zeros(B * _P * FREE, np.float32)
    v_p[dest] = vals
    v_p = v_p.reshape(B, _P, FREE).astype(ml_dtypes.float8_e4m3)

    # meta: recip [0:4], w2 [4:8], ones col 8, -ones col 9
    cnt = np.zeros((B, _P, nslot), np.float64)
    for b in range(B):
        cnt[b, part[b], slot[b]] = lens2[b]
    cmax = np.maximum(cnt, 1.0)
    recip = 1.0 / cmax
    w2 = np.asarray(Ls)[None, None, :] - cnt  # npad per (partition, slot)
    meta = np.zeros((B, _P, 16), np.float32)
    meta[:, :, 0:nslot] = recip
    meta[:, :, 4:4 + nslot] = w2
    meta[:, :, 8] = 1.0
    meta[:, :, 9] = -1.0
    return v_p, meta, Ls


def kernel(input, rows, cols, seg_ids, _trace=False, _num_paths=_NUM_PATHS,
           _nact=_NACT):
    from concourse.bass_utils import run_bass_kernel_spmd

    input = np.ascontiguousarray(np.asarray(input, np.float32))
    rows = np.ascontiguousarray(np.asarray(rows, np.int32))
    cols = np.ascontiguousarray(np.asarray(cols, np.int32))
    seg_ids = np.ascontiguousarray(np.asarray(seg_ids, np.int32))
    B = input.shape[0]

    v_p, meta, Ls = _pack(input, rows, cols, seg_ids, _num_paths)
    nc = _get_nc((Ls, _nact))
    in_maps = [{"vP": v_p[i], "meta": meta[i]} for i in range(B)]
    res = run_bass_kernel_spmd(nc, in_maps, core_ids=list(range(B)),
                               trace=_trace)
    total = sum(float(r["out"][0, 0]) for r in res.results)
    out = np.float32(total / B)
    if _trace:
        return out, res
    return out
